# revision 10
# baseline (speedup 1.0000x reference)
"""DTNNStep graph-message-passing kernel for 8x Trainium2 NeuronCores.

Strategy: distance_membership_i is sorted, so pairs are sharded by
destination-atom range (6250 atoms per core -> contiguous pair range per
core). Each core processes its pairs in 128-atom "windows"; within a
window, pairs are padded to a fixed capacity (TPW tiles of 128) so the
instruction stream is identical across cores (SPMD). The segment sum is
a matmul with a one-hot selection matrix generated on-device from
host-precomputed window-relative indices. No collectives are needed:
each core owns a disjoint slice of the output.

Per core:
  phase A: afh = atom_features @ W_cf + b_cf  -> DRAM table (gather source)
  phase B: per window w (49 total), per pair-tile k (18 per window):
      dh    = distT_ext[:,tile].T @ Wdf_ext          (bias via ones-row)
      G     = afh[j[tile]]                           (indirect DMA gather)
      fused = dh * G
      msgs  = tanh((fused)^T^T @ W_fc)               (PE transpose + matmul)
      S     = onehot(i - window_base)                (iota == i', DVE)
      win  += S.T @ msgs                             (PSUM accumulation)
    flush: out[window] = win - tanh((b_df*afh_own) @ W_fc) + af[window]
"""

import sys

for _p in ("/opt/trn_rl_repo",):
    if _p not in sys.path:
        sys.path.insert(0, _p)

import numpy as np
import concourse.bass as bass
import concourse.bacc as bacc
import concourse.tile as tile
from concourse import mybir
from concourse.bass_utils import run_bass_kernel_spmd

F32 = mybir.dt.float32
I32 = mybir.dt.int32

P = 128
N_ATOMS = 50000
N_PAIRS = 800000
N_EMB = 128
N_DIST = 100
N_HID = 128
NCORES = 8
APC = N_ATOMS // NCORES            # atoms per core: 6250
NWIN = (APC + P - 1) // P          # windows per core: 49
APC_PAD = NWIN * P                 # 6272
TPW = 18                           # pair tiles per window
CAP = TPW * P                      # pair capacity per window: 2304
NTBL = 50176                       # afh table rows (50000 padded to 98*512)
TBL_CH = NTBL // 512               # phase-A chunks: 98
CPACK_W = 8 * P                    # packed constants width


def build_nc():
    nc = bacc.Bacc()

    distT = nc.declare_dram_parameter("distT", [101, NWIN * CAP], F32, isOutput=False)
    jidx = nc.declare_dram_parameter("jidx", [NWIN, P, TPW], I32, isOutput=False)
    iprime = nc.declare_dram_parameter("iprime", [NWIN, P, TPW], F32, isOutput=False)
    afT = nc.declare_dram_parameter("afT", [P, NTBL], F32, isOutput=False)
    af_own = nc.declare_dram_parameter("af_own", [APC_PAD, P], F32, isOutput=False)
    afT_own = nc.declare_dram_parameter("afT_own", [P, APC_PAD], F32, isOutput=False)
    cpack_d = nc.declare_dram_parameter("cpack", [P, CPACK_W], F32, isOutput=False)
    out_d = nc.declare_dram_parameter("out", [APC_PAD, P], F32, isOutput=True)

    with tile.TileContext(nc) as tc:
        with (
            tc.tile_pool(name="dramtbl", bufs=1, space="DRAM") as tbl_pool,
            tc.tile_pool(name="consts", bufs=1) as cpool,
            tc.tile_pool(name="aft", bufs=3) as aft_pool,
            tc.tile_pool(name="afh", bufs=3) as afh_pool,
            tc.tile_pool(name="dist", bufs=2) as dist_pool,
            tc.tile_pool(name="gth", bufs=2) as gth_pool,
            tc.tile_pool(name="idx", bufs=2) as idx_pool,
            tc.tile_pool(name="fused", bufs=2) as fused_pool,
            tc.tile_pool(name="fusedT", bufs=2) as fusedT_pool,
            tc.tile_pool(name="msgs_sb", bufs=2) as msgs_sb_pool,
            tc.tile_pool(name="sgen", bufs=4) as s_pool,
            tc.tile_pool(name="flush", bufs=2) as fl_pool,
            tc.tile_pool(name="ps_dh", bufs=2, space="PSUM") as dh_ps,
            tc.tile_pool(name="ps_tp", bufs=2, space="PSUM") as tp_ps,
            tc.tile_pool(name="ps_msgs", bufs=2, space="PSUM") as msgs_ps,
            tc.tile_pool(name="ps_win", bufs=2, space="PSUM") as win_ps,
        ):
            table = tbl_pool.tile([NTBL, P], F32)

            # ---- constants: one packed tile, one DMA, one semaphore ----
            cpk = cpool.tile([P, CPACK_W], F32)
            nc.sync.dma_start(cpk[:], cpack_d[:])
            wcf = cpk[:, 0:P]
            wdfe = cpk[:101, P:2 * P]
            wfc = cpk[:, 2 * P:3 * P]
            iota = cpk[:, 3 * P:4 * P]
            ident = cpk[:, 4 * P:5 * P]
            bdf = cpk[:, 5 * P:5 * P + 1]
            bcf = cpk[0:1, 5 * P + 1:6 * P + 1]
            ones = cpk[0:1, 6 * P + 1:7 * P + 1]

            # ---- phase A: afh table = af @ W_cf + b_cf ----
            for ch in range(TBL_CH):
                a = aft_pool.tile([P, 512], F32)
                nc.sync.dma_start(a[:], afT[:, ch * 512:(ch + 1) * 512])
                ps = dh_ps.tile([P, 512], F32, tag="dh")
                for s in range(4):
                    reg = ps[:, s * P:(s + 1) * P]
                    nc.tensor.matmul(reg, lhsT=a[:, s * P:(s + 1) * P],
                                     rhs=wcf, start=True, stop=False)
                    nc.tensor.matmul(reg, lhsT=ones, rhs=bcf,
                                     start=False, stop=True)
                o = afh_pool.tile([P, 512], F32)
                if ch % 2 == 0:
                    nc.vector.tensor_copy(o[:], ps[:])
                else:
                    nc.scalar.copy(o[:], ps[:])
                dst = table[ch * 512:(ch + 1) * 512, :].rearrange(
                    "(s p) h -> p s h", p=P)
                nc.sync.dma_start(dst, o[:].rearrange("p (s h) -> p s h", h=P))

            tc.strict_bb_all_engine_barrier()

            # ---- phase B: main pair loop ----
            for w in range(NWIN):
                jt = idx_pool.tile([P, TPW], I32, tag="jt")
                nc.sync.dma_start(jt[:], jidx[w])
                it = idx_pool.tile([P, TPW], F32, tag="it")
                nc.sync.dma_start(it[:], iprime[w])
                dt = dist_pool.tile([101, CAP], F32)
                nc.sync.dma_start(dt[:], distT[:, w * CAP:(w + 1) * CAP])
                gt = gth_pool.tile([P, TPW * P], F32)
                for k in range(TPW):
                    nc.gpsimd.indirect_dma_start(
                        out=gt[:, k * P:(k + 1) * P],
                        out_offset=None,
                        in_=table[:],
                        in_offset=bass.IndirectOffsetOnAxis(
                            ap=jt[:, k:k + 1], axis=0),
                    )
                win = win_ps.tile([P, P], F32)

                k = 0
                while k < TPW:
                    nblk = min(4, TPW - k)
                    nb = nblk * P
                    dh = dh_ps.tile([P, 512], F32, tag="dh")
                    for s in range(nblk):
                        nc.tensor.matmul(
                            dh[:, s * P:(s + 1) * P],
                            lhsT=dt[:, (k + s) * P:(k + s + 1) * P],
                            rhs=wdfe, start=True, stop=True)
                    fused = fused_pool.tile([P, 512], F32)
                    nc.vector.tensor_tensor(
                        fused[:, :nb], dh[:, :nb],
                        gt[:, k * P:k * P + nb], op=mybir.AluOpType.mult)
                    tp = tp_ps.tile([P, 512], F32)
                    for s in range(nblk):
                        nc.tensor.transpose(
                            tp[:, s * P:(s + 1) * P],
                            fused[:, s * P:(s + 1) * P], ident)
                    fusedT = fusedT_pool.tile([P, 512], F32)
                    nc.scalar.copy(fusedT[:, :nb], tp[:, :nb])
                    mps = msgs_ps.tile([P, 512], F32, tag="mps")
                    for s in range(nblk):
                        nc.tensor.matmul(
                            mps[:, s * P:(s + 1) * P],
                            lhsT=fusedT[:, s * P:(s + 1) * P],
                            rhs=wfc, start=True, stop=True)
                    msgs = msgs_sb_pool.tile([P, 512], F32)
                    nc.scalar.activation(msgs[:, :nb], mps[:, :nb],
                                         mybir.ActivationFunctionType.Tanh)
                    for s in range(nblk):
                        kk = k + s
                        S = s_pool.tile([P, P], F32)
                        nc.vector.tensor_scalar(
                            out=S[:], in0=iota, scalar1=it[:, kk:kk + 1],
                            scalar2=None, op0=mybir.AluOpType.is_equal)
                        nc.tensor.matmul(
                            win[:], lhsT=S[:], rhs=msgs[:, s * P:(s + 1) * P],
                            start=(kk == 0), stop=(kk == TPW - 1))
                    k += nblk

                # ---- window flush ----
                afTo = fl_pool.tile([P, P], F32, tag="afTo")
                nc.sync.dma_start(afTo[:], afT_own[:, w * P:(w + 1) * P])
                afo = fl_pool.tile([P, P], F32, tag="afo")
                nc.sync.dma_start(afo[:], af_own[w * P:(w + 1) * P, :])
                ah = dh_ps.tile([P, P], F32, tag="dh")
                nc.tensor.matmul(ah[:], lhsT=wcf, rhs=afTo[:],
                                 start=True, stop=False)
                nc.tensor.matmul(ah[:], lhsT=bcf, rhs=ones,
                                 start=False, stop=True)
                iipre = fl_pool.tile([P, P], F32, tag="iipre")
                nc.vector.tensor_scalar(
                    out=iipre[:], in0=ah[:], scalar1=bdf, scalar2=None,
                    op0=mybir.AluOpType.mult)
                iips = msgs_ps.tile([P, P], F32, tag="mps")
                nc.tensor.matmul(iips[:], lhsT=iipre[:], rhs=wfc,
                                 start=True, stop=True)
                ii = fl_pool.tile([P, P], F32, tag="ii")
                nc.scalar.activation(ii[:], iips[:],
                                     mybir.ActivationFunctionType.Tanh)
                tmp = fl_pool.tile([P, P], F32, tag="tmp")
                nc.vector.tensor_tensor(tmp[:], win[:], ii[:],
                                        op=mybir.AluOpType.subtract)
                res = fl_pool.tile([P, P], F32, tag="res")
                nc.vector.tensor_tensor(res[:], tmp[:], afo[:],
                                        op=mybir.AluOpType.add)
                nc.sync.dma_start(out_d[w * P:(w + 1) * P, :], res[:])

    nc.compile()
    return nc


def host_prep(atom_features, distance, atom_membership,
              distance_membership_i, distance_membership_j,
              W_cf, W_df, W_fc, b_cf, b_df):
    af = np.ascontiguousarray(atom_features, dtype=np.float32)
    dist = np.ascontiguousarray(distance, dtype=np.float32)
    i = np.ascontiguousarray(distance_membership_i, dtype=np.int64)
    j = np.ascontiguousarray(distance_membership_j, dtype=np.int32)

    afT_full = np.zeros((P, NTBL), np.float32)
    afT_full[:, :N_ATOMS] = af.T
    wdfe = np.concatenate([np.asarray(W_df, np.float32),
                           np.asarray(b_df, np.float32)[None, :]], axis=0)
    iota = np.tile(np.arange(P, dtype=np.float32)[None, :], (P, 1))
    ident = np.eye(P, dtype=np.float32)
    cpack = np.zeros((P, CPACK_W), np.float32)
    cpack[:, 0:P] = np.asarray(W_cf, np.float32)
    cpack[:101, P:2 * P] = wdfe
    cpack[:, 2 * P:3 * P] = np.asarray(W_fc, np.float32)
    cpack[:, 3 * P:4 * P] = iota
    cpack[:, 4 * P:5 * P] = ident
    cpack[:, 5 * P] = np.asarray(b_df, np.float32)
    cpack[0, 5 * P + 1:6 * P + 1] = np.asarray(b_cf, np.float32)
    cpack[0, 6 * P + 1:7 * P + 1] = 1.0
    shared = {
        "afT": afT_full,
        "cpack": cpack,
    }

    in_maps = []
    for c in range(NCORES):
        distT_c = np.zeros((101, NWIN * CAP), np.float32)
        distT_c[100, :] = 1.0
        j_c = np.zeros((NWIN, P, TPW), np.int32)
        ip_c = np.full((NWIN, P, TPW), -1.0, np.float32)
        for w in range(NWIN):
            B = c * APC + w * P
            E = min(B + P, (c + 1) * APC)
            pb = int(np.searchsorted(i, B))
            pe = int(np.searchsorted(i, E))
            n = pe - pb
            if n > CAP:
                raise AssertionError(f"window overflow: {n} > {CAP}")
            col0 = w * CAP
            distT_c[:100, col0:col0 + n] = dist[pb:pe].T
            jw = np.zeros(CAP, np.int32)
            jw[:n] = j[pb:pe]
            ipw = np.full(CAP, -1.0, np.float32)
            ipw[:n] = (i[pb:pe] - B).astype(np.float32)
            j_c[w] = jw.reshape(TPW, P).T
            ip_c[w] = ipw.reshape(TPW, P).T
        af_own = np.zeros((APC_PAD, P), np.float32)
        af_own[:APC] = af[c * APC:(c + 1) * APC]
        m = {
            "distT": distT_c,
            "jidx": j_c,
            "iprime": ip_c,
            "af_own": af_own,
            "afT_own": np.ascontiguousarray(af_own.T),
        }
        m.update(shared)
        in_maps.append(m)
    return in_maps


_NC_CACHE = {}


def get_nc():
    if "nc" not in _NC_CACHE:
        _NC_CACHE["nc"] = build_nc()
    return _NC_CACHE["nc"]


def kernel(**inputs):
    in_maps = host_prep(**inputs)
    nc = get_nc()
    res = run_bass_kernel_spmd(nc, in_maps, core_ids=list(range(NCORES)))
    out = np.empty((N_ATOMS, N_EMB), np.float32)
    for c in range(NCORES):
        out[c * APC:(c + 1) * APC] = res.results[c]["out"][:APC]
    return out


# revision 16
# speedup vs baseline: 1.5830x; 1.5830x over previous
"""DTNNStep graph-message-passing kernel for 8x Trainium2 NeuronCores.

Strategy: distance_membership_i is sorted, so pairs are sharded by
destination-atom range (6250 atoms per core -> contiguous pair range per
core). Each core processes its pairs in 128-atom "windows"; within a
window, pairs are padded to a fixed capacity (TPW tiles of 128) so the
instruction stream is identical across cores (SPMD). The segment sum is
a matmul with a one-hot selection matrix generated on-device from
host-precomputed window-relative indices. No collectives are needed:
each core owns a disjoint slice of the output.

The afh gather table lives in DRAM with two zero rows; the per-pair
gather afh[j] runs as two dma_gather ops per window (int16 indices are
signed, so j is split at 32256 with out-of-range slots pointing at a
zero row) summed on DVE. The matmul value path runs in bf16; segment
accumulation stays in f32 PSUM.
"""

import sys

for _p in ("/opt/trn_rl_repo",):
    if _p not in sys.path:
        sys.path.insert(0, _p)

import numpy as np
import ml_dtypes
import concourse.bass as bass
import concourse.bacc as bacc
import concourse.tile as tile
from concourse import mybir
from concourse.bass_utils import run_bass_kernel_spmd

F32 = mybir.dt.float32
BF16 = mybir.dt.bfloat16
I16 = mybir.dt.int16
NPBF = ml_dtypes.bfloat16

P = 128
N_ATOMS = 50000
N_PAIRS = 800000
N_EMB = 128
NCORES = 8
APC = N_ATOMS // NCORES            # atoms per core: 6250
NWIN = (APC + P - 1) // P          # windows per core: 49
APC_PAD = NWIN * P                 # 6272
TPW = 18                           # pair tiles per window
CAP = TPW * P                      # pair capacity per window: 2304
NTBL = 50176                       # table rows (50002 used, padded)
TBL_CH = NTBL // 512               # phase-A chunks: 98
SPLIT = 63 * 512                   # 32256: j >= SPLIT served by gather B
C16W = 5 * P                       # bf16 const pack width
C32W = 3 * P                       # f32 const pack width


def build_nc():
    nc = bacc.Bacc()

    distT = nc.declare_dram_parameter("distT", [101, NWIN * CAP], BF16,
                                      isOutput=False)
    jidxa = nc.declare_dram_parameter("jidxa", [NWIN, P, CAP // 16], I16,
                                      isOutput=False)
    jidxb = nc.declare_dram_parameter("jidxb", [NWIN, P, CAP // 16], I16,
                                      isOutput=False)
    iprime = nc.declare_dram_parameter("iprime", [NWIN, P, TPW], F32,
                                       isOutput=False)
    afT = nc.declare_dram_parameter("afT", [P, NTBL], BF16, isOutput=False)
    af_own = nc.declare_dram_parameter("af_own", [APC_PAD, P], F32,
                                       isOutput=False)
    afT_own = nc.declare_dram_parameter("afT_own", [P, APC_PAD], BF16,
                                        isOutput=False)
    cp16_d = nc.declare_dram_parameter("cp16", [P, C16W], BF16, isOutput=False)
    cp32_d = nc.declare_dram_parameter("cp32", [P, C32W], F32, isOutput=False)
    out_d = nc.declare_dram_parameter("out", [APC_PAD, P], F32, isOutput=True)

    with tile.TileContext(nc) as tc:
        with (
            tc.tile_pool(name="dramtbl", bufs=1, space="DRAM") as tbl_pool,
            tc.tile_pool(name="consts", bufs=1) as cpool,
            tc.tile_pool(name="aft", bufs=4) as aft_pool,
            tc.tile_pool(name="afh", bufs=4) as afh_pool,
            tc.tile_pool(name="dist", bufs=3) as dist_pool,
            tc.tile_pool(name="gth", bufs=3) as gth_pool,
            tc.tile_pool(name="idx", bufs=3) as idx_pool,
            tc.tile_pool(name="fused", bufs=3) as fused_pool,
            tc.tile_pool(name="fusedT", bufs=3) as fusedT_pool,
            tc.tile_pool(name="msgs_sb", bufs=3) as msgs_sb_pool,
            tc.tile_pool(name="sgen", bufs=6) as s_pool,
            tc.tile_pool(name="flush", bufs=3) as fl_pool,
            tc.tile_pool(name="ps_dh", bufs=2, space="PSUM") as dh_ps,
            tc.tile_pool(name="ps_tp", bufs=2, space="PSUM") as tp_ps,
            tc.tile_pool(name="ps_msgs", bufs=2, space="PSUM") as msgs_ps,
            tc.tile_pool(name="ps_win", bufs=2, space="PSUM") as win_ps,
        ):
            table = tbl_pool.tile([NTBL + 2, P], BF16)

            cpk = cpool.tile([P, C16W], BF16)
            nc.sync.dma_start(cpk[:], cp16_d[:])
            wcf = cpk[:, 0:P]
            wdfe = cpk[:101, P:2 * P]
            wfc = cpk[:, 2 * P:3 * P]
            iota = cpk[:, 3 * P:4 * P]
            ident = cpk[:, 4 * P:5 * P]
            cpk32 = cpool.tile([P, C32W], F32)
            nc.sync.dma_start(cpk32[:], cp32_d[:])
            bdf = cpk32[:, 0:1]
            bcf = cpk32[0:1, P:2 * P]
            ones = cpk32[0:1, 2 * P:3 * P]

            # zero rows of the gather table (rows 0 and SPLIT+1)
            zrow = cpool.tile([1, P], BF16)
            nc.gpsimd.memset(zrow[:], 0.0)
            nc.sync.dma_start(table[0:1, :], zrow[:])
            nc.sync.dma_start(table[SPLIT + 1:SPLIT + 2, :], zrow[:])

            # ---- phase A: afh table = af @ W_cf + b_cf (bf16, shifted) ----
            for ch in range(TBL_CH):
                a = aft_pool.tile([P, 512], BF16)
                nc.sync.dma_start(a[:], afT[:, ch * 512:(ch + 1) * 512])
                ps = dh_ps.tile([P, 512], F32, tag="dh")
                for s in range(4):
                    reg = ps[:, s * P:(s + 1) * P]
                    nc.tensor.matmul(reg, lhsT=a[:, s * P:(s + 1) * P],
                                     rhs=wcf, start=True, stop=False)
                    nc.tensor.matmul(reg, lhsT=ones, rhs=bcf,
                                     start=False, stop=True)
                o = afh_pool.tile([P, 512], BF16)
                if ch % 2 == 0:
                    nc.vector.tensor_copy(o[:], ps[:])
                else:
                    nc.scalar.copy(o[:], ps[:])
                r0 = ch * 512 + (1 if ch < 63 else 2)
                dst = table[r0:r0 + 512, :].rearrange("(s p) h -> p s h", p=P)
                nc.sync.dma_start(dst, o[:].rearrange("p (s h) -> p s h", h=P))

            tc.strict_bb_all_engine_barrier()

            # ---- phase B: main pair loop ----
            for w in range(NWIN):
                ixa = idx_pool.tile([P, CAP // 16], I16, tag="ixa")
                nc.sync.dma_start(ixa[:], jidxa[w])
                ixb = idx_pool.tile([P, CAP // 16], I16, tag="ixb")
                nc.sync.dma_start(ixb[:], jidxb[w])
                it = idx_pool.tile([P, TPW], F32, tag="it")
                nc.sync.dma_start(it[:], iprime[w])
                dt = dist_pool.tile([101, CAP], BF16)
                nc.sync.dma_start(dt[:], distT[:, w * CAP:(w + 1) * CAP])

                ga = gth_pool.tile([P, TPW * P], BF16, tag="ga")
                nc.gpsimd.dma_gather(
                    out_ap=ga[:].rearrange("p (k h) -> p k h", h=P),
                    in_ap=table[:],
                    idxs_ap=ixa[:], num_idxs=CAP, num_idxs_reg=CAP,
                    elem_size=P)
                gb = gth_pool.tile([P, TPW * P], BF16, tag="gb")
                nc.gpsimd.dma_gather(
                    out_ap=gb[:].rearrange("p (k h) -> p k h", h=P),
                    in_ap=table[SPLIT + 1:, :],
                    idxs_ap=ixb[:], num_idxs=CAP, num_idxs_reg=CAP,
                    elem_size=P)
                gt = gth_pool.tile([P, TPW * P], BF16, tag="gt")
                nc.vector.tensor_tensor(gt[:], ga[:], gb[:],
                                        op=mybir.AluOpType.add)

                win = win_ps.tile([P, P], F32)

                k = 0
                blk = 0
                while k < TPW:
                    nblk = min(4, TPW - k)
                    nb = nblk * P
                    dh = dh_ps.tile([P, 512], F32, tag="dh")
                    for s in range(nblk):
                        nc.tensor.matmul(
                            dh[:, s * P:(s + 1) * P],
                            lhsT=dt[:, (k + s) * P:(k + s + 1) * P],
                            rhs=wdfe, start=True, stop=True)
                    fused = fused_pool.tile([P, 512], BF16)
                    nc.vector.tensor_tensor(
                        fused[:, :nb], dh[:, :nb],
                        gt[:, k * P:k * P + nb], op=mybir.AluOpType.mult)
                    tp = tp_ps.tile([P, 512], BF16)
                    for s in range(nblk):
                        nc.tensor.transpose(
                            tp[:, s * P:(s + 1) * P],
                            fused[:, s * P:(s + 1) * P], ident)
                    fusedT = fusedT_pool.tile([P, 512], BF16)
                    if blk % 2 == 0:
                        nc.scalar.copy(fusedT[:, :nb], tp[:, :nb])
                    else:
                        nc.vector.tensor_copy(fusedT[:, :nb], tp[:, :nb])
                    mps = msgs_ps.tile([P, 512], F32, tag="mps")
                    for s in range(nblk):
                        nc.tensor.matmul(
                            mps[:, s * P:(s + 1) * P],
                            lhsT=fusedT[:, s * P:(s + 1) * P],
                            rhs=wfc, start=True, stop=True)
                    msgs = msgs_sb_pool.tile([P, 512], BF16)
                    nc.scalar.activation(msgs[:, :nb], mps[:, :nb],
                                         mybir.ActivationFunctionType.Tanh)
                    for s in range(nblk):
                        kk = k + s
                        S = s_pool.tile([P, P], BF16)
                        nc.vector.tensor_scalar(
                            out=S[:], in0=iota, scalar1=it[:, kk:kk + 1],
                            scalar2=None, op0=mybir.AluOpType.is_equal)
                        nc.tensor.matmul(
                            win[:], lhsT=S[:], rhs=msgs[:, s * P:(s + 1) * P],
                            start=(kk == 0), stop=(kk == TPW - 1))
                    k += nblk
                    blk += 1

                # ---- window flush ----
                afTo = fl_pool.tile([P, P], BF16, tag="afTo")
                nc.sync.dma_start(afTo[:], afT_own[:, w * P:(w + 1) * P])
                afo = fl_pool.tile([P, P], F32, tag="afo")
                nc.sync.dma_start(afo[:], af_own[w * P:(w + 1) * P, :])
                ah = dh_ps.tile([P, P], F32, tag="dh")
                nc.tensor.matmul(ah[:], lhsT=wcf, rhs=afTo[:],
                                 start=True, stop=False)
                nc.tensor.matmul(ah[:], lhsT=bcf, rhs=ones,
                                 start=False, stop=True)
                iipre = fl_pool.tile([P, P], BF16, tag="iipre")
                nc.vector.tensor_scalar(
                    out=iipre[:], in0=ah[:], scalar1=bdf, scalar2=None,
                    op0=mybir.AluOpType.mult)
                iips = msgs_ps.tile([P, P], F32, tag="mps")
                nc.tensor.matmul(iips[:], lhsT=iipre[:], rhs=wfc,
                                 start=True, stop=True)
                ii = fl_pool.tile([P, P], F32, tag="ii")
                nc.scalar.activation(ii[:], iips[:],
                                     mybir.ActivationFunctionType.Tanh)
                tmp = fl_pool.tile([P, P], F32, tag="tmp")
                nc.vector.tensor_tensor(tmp[:], win[:], ii[:],
                                        op=mybir.AluOpType.subtract)
                res = fl_pool.tile([P, P], F32, tag="res")
                nc.vector.tensor_tensor(res[:], tmp[:], afo[:],
                                        op=mybir.AluOpType.add)
                nc.sync.dma_start(out_d[w * P:(w + 1) * P, :], res[:])

    nc.compile()
    return nc


def _wrap16(ix):
    """idx n -> [n % 16, n // 16], replicated to 128 partitions."""
    a = np.ascontiguousarray(ix.reshape(-1, 16).T)          # [16, CAP//16]
    return np.tile(a, (8, 1))                               # [128, CAP//16]


def host_prep(atom_features, distance, atom_membership,
              distance_membership_i, distance_membership_j,
              W_cf, W_df, W_fc, b_cf, b_df):
    af = np.ascontiguousarray(atom_features, dtype=np.float32)
    dist = np.ascontiguousarray(distance, dtype=np.float32)
    i = np.ascontiguousarray(distance_membership_i, dtype=np.int64)
    j = np.ascontiguousarray(distance_membership_j, dtype=np.int64)

    afT_full = np.zeros((P, NTBL), NPBF)
    afT_full[:, :N_ATOMS] = af.T.astype(NPBF)
    wdfe = np.concatenate([np.asarray(W_df, np.float32),
                           np.asarray(b_df, np.float32)[None, :]], axis=0)
    cp16 = np.zeros((P, C16W), np.float32)
    cp16[:, 0:P] = np.asarray(W_cf, np.float32)
    cp16[:101, P:2 * P] = wdfe
    cp16[:, 2 * P:3 * P] = np.asarray(W_fc, np.float32)
    cp16[:, 3 * P:4 * P] = np.arange(P, dtype=np.float32)[None, :]
    cp16[:, 4 * P:5 * P] = np.eye(P, dtype=np.float32)
    cp32 = np.zeros((P, C32W), np.float32)
    cp32[:, 0] = np.asarray(b_df, np.float32)
    cp32[0, P:2 * P] = np.asarray(b_cf, np.float32)
    cp32[0, 2 * P:3 * P] = 1.0
    shared = {
        "afT": afT_full,
        "cp16": cp16.astype(NPBF),
        "cp32": cp32,
    }

    in_maps = []
    for c in range(NCORES):
        distT_c = np.zeros((101, NWIN * CAP), NPBF)
        distT_c[100, :] = 1.0
        ja_c = np.zeros((NWIN, P, CAP // 16), np.int16)
        jb_c = np.zeros((NWIN, P, CAP // 16), np.int16)
        ip_c = np.full((NWIN, P, TPW), -1.0, np.float32)
        for w in range(NWIN):
            B = c * APC + w * P
            E = min(B + P, (c + 1) * APC)
            pb = int(np.searchsorted(i, B))
            pe = int(np.searchsorted(i, E))
            n = pe - pb
            if n > CAP:
                raise AssertionError(f"window overflow: {n} > {CAP}")
            col0 = w * CAP
            distT_c[:100, col0:col0 + n] = dist[pb:pe].T.astype(NPBF)
            jw = np.zeros(CAP, np.int64)
            jw[:n] = j[pb:pe]
            ja_c[w] = _wrap16(np.where(jw < SPLIT, jw + 1, 0).astype(np.int16))
            jb_c[w] = _wrap16(
                np.where(jw >= SPLIT, jw - SPLIT + 1, 0).astype(np.int16))
            ipw = np.full(CAP, -1.0, np.float32)
            ipw[:n] = (i[pb:pe] - B).astype(np.float32)
            ip_c[w] = ipw.reshape(TPW, P).T
        af_own = np.zeros((APC_PAD, P), np.float32)
        af_own[:APC] = af[c * APC:(c + 1) * APC]
        m = {
            "distT": distT_c,
            "jidxa": ja_c,
            "jidxb": jb_c,
            "iprime": ip_c,
            "af_own": af_own,
            "afT_own": np.ascontiguousarray(af_own.T).astype(NPBF),
        }
        m.update(shared)
        in_maps.append(m)
    return in_maps


_NC_CACHE = {}


def get_nc():
    if "nc" not in _NC_CACHE:
        _NC_CACHE["nc"] = build_nc()
    return _NC_CACHE["nc"]


def kernel(**inputs):
    in_maps = host_prep(**inputs)
    nc = get_nc()
    res = run_bass_kernel_spmd(nc, in_maps, core_ids=list(range(NCORES)))
    out = np.empty((N_ATOMS, N_EMB), np.float32)
    for c in range(NCORES):
        out[c * APC:(c + 1) * APC] = res.results[c]["out"][:APC]
    return out
